# revision 25
# baseline (speedup 1.0000x reference)
"""Trainium2 Bass kernel for nn_Attention_59949153518227.

Dense transformer block: adaLN-style modulation -> per-stream QKV -> RoPE ->
shared MHA over concat(state, action) -> out_proj -> per-stream MLP with
residual scaling.  B=8 batch elements data-parallel across 8 NeuronCores.

Per-core dataflow (feature-on-partition layout [128p, tiles, tokens]):
  z[t,d] --PE transpose--> zT[d,t] --(1+scale)*z+shift--> xT
  xT --matmul wqkvT--> q,k,v  (q,k rows pre-permuted even/odd for RoPE)
  rope(q), rope(k) in-place (elementwise, cos/sin tables from host)
  q' = wq.T@q (1/8 folded), k' = wk.T@k   [e', t]
  v' = v.T@wvT + bv  [t, e'] natural, packed per-head with a ones column
  scores sT[k,q] = k'_h.T @ q'_h ; p = exp(sT) ; o_h = [v_h|1].T @ p
  row 64 of o = softmax denominator; raw o stored, denominators batched into
  one [32,512] tile -> single reciprocal -> per-head rank-1 PE broadcast ->
  in-place normalize of oT[e,t]
  y = wo.T@o + bo ; h = gelu(w1.T@y + b1) ; down = h.T@w2T + b2 (rank-1)
  out = z + down * residual  (z kept fp32; residual broadcast from host)

Matmul dtype is DTM (bfloat16 by default; float32r fallback), fp32 PSUM.
"""
import math
import sys

import numpy as np

try:
    import concourse.bass as bass  # noqa: F401
except ImportError:  # pragma: no cover
    sys.path.insert(0, "/opt/trn_rl_repo")

import ml_dtypes
import concourse.bass as bass
import concourse.tile as tile
from concourse import bacc, mybir
from concourse.bass_utils import run_bass_kernel_spmd

F32 = mybir.dt.float32
F32R = mybir.dt.float32r
BF16 = mybir.dt.bfloat16
AF = mybir.ActivationFunctionType
OP = mybir.AluOpType

DTM = BF16                      # matmul-side dtype knob: BF16 or F32R
NPM = ml_dtypes.bfloat16 if DTM == BF16 else np.float32

B, S, D, H, HD = 8, 512, 1024, 16, 64
T = 2 * S
FF = 4 * D
P = 128
MAX_LEN = 512.0
N_CORES = 8

_BUILD_CACHE = {}


def _build_nc():
    nc = bacc.Bacc()

    # ---- per-core data inputs ----
    sz_d = nc.dram_tensor("sz", [S, D], F32, kind="ExternalInput")
    az_d = nc.dram_tensor("az", [S, D], F32, kind="ExternalInput")
    sc1p_d = nc.dram_tensor("scale1p", [P, 8], F32, kind="ExternalInput")
    shift_d = nc.dram_tensor("shift", [P, 8], F32, kind="ExternalInput")
    resb_d = nc.dram_tensor("resb", [P, D], F32, kind="ExternalInput")

    # ---- shared weights/constants (replicated to all cores) ----
    wqkv_d = [nc.dram_tensor(f"wqkv{s}", [24, P, 8, P], DTM, kind="ExternalInput") for s in range(2)]
    bqkv_d = [nc.dram_tensor(f"bqkv{s}", [P, 24], F32, kind="ExternalInput") for s in range(2)]
    wq_d = nc.dram_tensor("wq", [8, P, 8, P], DTM, kind="ExternalInput")
    wk_d = nc.dram_tensor("wk", [8, P, 8, P], DTM, kind="ExternalInput")
    bq_d = nc.dram_tensor("bq", [P, 8], F32, kind="ExternalInput")
    bk_d = nc.dram_tensor("bk", [P, 8], F32, kind="ExternalInput")
    wvT_d = nc.dram_tensor("wvT", [D, D], DTM, kind="ExternalInput")
    bvrow_d = nc.dram_tensor("bvrow", [1, D], DTM, kind="ExternalInput")
    wo_d = nc.dram_tensor("wo", [8, P, 8, P], DTM, kind="ExternalInput")
    bo_d = nc.dram_tensor("bo", [P, 8], F32, kind="ExternalInput")
    w1_d = [nc.dram_tensor(f"w1{s}", [32, P, 8, P], DTM, kind="ExternalInput") for s in range(2)]
    b1_d = [nc.dram_tensor(f"b1{s}", [P, 32], F32, kind="ExternalInput") for s in range(2)]
    w2T_d = [nc.dram_tensor(f"w2T{s}", [FF, D], DTM, kind="ExternalInput") for s in range(2)]
    b2row_d = [nc.dram_tensor(f"b2row{s}", [1, D], DTM, kind="ExternalInput") for s in range(2)]
    cos_d = nc.dram_tensor("cosT", [P, 2048], DTM, kind="ExternalInput")
    sin_d = nc.dram_tensor("sinT", [P, 2048], DTM, kind="ExternalInput")
    ident_d = nc.dram_tensor("ident", [P, P], F32, kind="ExternalInput")
    onesr_d = nc.dram_tensor("onesr", [1, P], DTM, kind="ExternalInput")
    onesc_d = nc.dram_tensor("onesc", [P, 16, 1], DTM, kind="ExternalInput")
    onesb_d = nc.dram_tensor("onesb", [P, 512], DTM, kind="ExternalInput")

    out_d = [nc.dram_tensor(n, [S, D], F32, kind="ExternalOutput") for n in ("state_out", "action_out")]

    z_src = [sz_d, az_d]

    with tile.TileContext(nc) as tc:
        with (
            tc.tile_pool(name="big", bufs=10) as big,
            tc.tile_pool(name="vsbp", bufs=2) as vsbp,
            tc.tile_pool(name="w1p", bufs=6) as w1p,
            tc.tile_pool(name="w2p", bufs=6) as w2p,
            tc.tile_pool(name="small", bufs=1) as small,
            tc.tile_pool(name="rs", bufs=6) as rs,
            tc.tile_pool(name="rbp", bufs=3) as rbp,
            tc.tile_pool(name="znp", bufs=2) as znp,
            tc.tile_pool(name="psum", bufs=1, space="PSUM") as psum,
        ):
            _ctr = [0]

            def _nm(pfx):
                _ctr[0] += 1
                return f"{pfx}{_ctr[0]}"

            def big_tile(shape):
                return big.tile(shape, DTM, tag="big", name=_nm("bigt"))

            def ps_mm():
                return psum.tile([P, 512], F32, tag="mm", bufs=2, name=_nm("psmm"))

            def copy_bias(dst, ps, bias_ap):
                # psum -> sbuf with per-partition bias add, on DVE
                nc.vector.tensor_scalar(out=dst, in0=ps, scalar1=bias_ap, scalar2=None, op0=OP.add)

            # z tiles DMA'd first (they gate the very first PE transposes)
            zn_t = []
            for s in range(2):
                z_t = znp.tile([P, 4, D], F32, tag="zn", name=_nm("zn"))
                nc.sync.dma_start(out=z_t[:], in_=z_src[s].rearrange("(to p) d -> p to d", p=P))
                zn_t.append(z_t)

            # ---- constants ----
            ident = small.tile([P, P], F32)
            nc.sync.dma_start(out=ident[:], in_=ident_d[:])
            onesr = small.tile([1, P], DTM)
            nc.sync.dma_start(out=onesr[:], in_=onesr_d[:])
            sc1p = small.tile([P, 8], F32)
            nc.sync.dma_start(out=sc1p[:], in_=sc1p_d[:])
            shf = small.tile([P, 8], F32)
            nc.sync.dma_start(out=shf[:], in_=shift_d[:])
            resb = small.tile([P, D], F32)
            nc.sync.dma_start(out=resb[:], in_=resb_d[:])
            bqkv = []
            for s in range(2):
                t_ = small.tile([P, 24], F32, tag=f"bqkv{s}")
                nc.sync.dma_start(out=t_[:], in_=bqkv_d[s][:])
                bqkv.append(t_)
            bq = small.tile([P, 8], F32, tag="bq")
            nc.sync.dma_start(out=bq[:], in_=bq_d[:])
            bk = small.tile([P, 8], F32, tag="bk")
            nc.sync.dma_start(out=bk[:], in_=bk_d[:])
            bo = small.tile([P, 8], F32, tag="bo")
            nc.sync.dma_start(out=bo[:], in_=bo_d[:])
            onesb = small.tile([P, 512], DTM, tag="onesb")
            nc.sync.dma_start(out=onesb[:], in_=onesb_d[:])
            bvrow = small.tile([1, D], DTM, tag="bvrow")
            nc.sync.dma_start(out=bvrow[:], in_=bvrow_d[:])
            b1 = []
            for s in range(2):
                t_ = small.tile([P, 32], F32, tag=f"b1{s}")
                nc.sync.dma_start(out=t_[:], in_=b1_d[s][:])
                b1.append(t_)
            b2row = []
            for s in range(2):
                t_ = small.tile([1, D], DTM, tag=f"b2row{s}")
                nc.sync.dma_start(out=t_[:], in_=b2row_d[s][:])
                b2row.append(t_)
            tbl = big_tile([P, 4096])
            nc.sync.dma_start(out=tbl[:, 0:2048], in_=cos_d[:])
            nc.sync.dma_start(out=tbl[:, 2048:4096], in_=sin_d[:])

            # v_sb allocated up-front so its ones-column DMAs land early in the
            # SP queue (they gate the psum-releasing copies in the v' stage).
            vsb = [vsbp.tile([P, 8, 8, 65], DTM, tag="vsb", name=_nm("vsb")) for _ in range(2)]
            for ec in range(2):
                for kt in range(8):
                    nc.sync.dma_start(out=vsb[ec][:, kt, :, 64:65], in_=onesc_d[:, ec * 8:(ec + 1) * 8, :])

            # ---- stages A+B interleaved per stream: transpose+modulate, qkv ----
            xT = [None, None]
            qkv = []  # [stream][j] j=0 q, 1 k, 2 v ; each [128, 8, 512]
            for s in range(2):
                z_t = zn_t[s]
                x_t = big_tile([P, 8, S])
                for do in range(8):
                    for to in range(4):
                        pt = psum.tile([P, P], F32, tag="mm", bufs=2, name=_nm("ptr"))
                        nc.tensor.transpose(pt[:], z_t[:, to, do * P:(do + 1) * P], ident[:])
                        nc.vector.tensor_scalar(
                            out=x_t[:, do, to * P:(to + 1) * P], in0=pt[:],
                            scalar1=sc1p[:, do:do + 1], scalar2=shf[:, do:do + 1],
                            op0=OP.mult, op1=OP.add)
                xT[s] = x_t
                parts = [big_tile([P, 8, S]) for _ in range(3)]
                for eo in range(24):
                    wt = w1p.tile([P, 8, P], DTM, tag="w1", name=_nm("wt"))
                    nc.sync.dma_start(out=wt[:], in_=wqkv_d[s][eo])
                    ps = ps_mm()
                    for ko in range(8):
                        nc.tensor.matmul(ps[:], lhsT=wt[:, ko, :], rhs=x_t[:, ko, :],
                                         start=(ko == 0), stop=(ko == 7))
                    j, col = divmod(eo, 8)
                    copy_bias(parts[j][:, col, :], ps[:], bqkv[s][:, eo:eo + 1])
                qkv.append(parts)

            # ---- stage C: rope on q and k blocks, in place ----
            for s in range(2):
                for j in range(2):
                    tgt = qkv[s][j]
                    for i in range(4):
                        qe = tgt[:, i, :]
                        qo = tgt[:, 4 + i, :]
                        cos_i = tbl[:, i * 512:(i + 1) * 512]
                        sin_i = tbl[:, 2048 + i * 512:2048 + (i + 1) * 512]
                        m1 = rs.tile([P, S], DTM, tag="rs", name=_nm("rst"))
                        m2 = rs.tile([P, S], DTM, tag="rs", name=_nm("rst"))
                        m3 = rs.tile([P, S], DTM, tag="rs", name=_nm("rst"))
                        m4 = rs.tile([P, S], DTM, tag="rs", name=_nm("rst"))
                        nc.vector.tensor_tensor(m1[:], qe, cos_i, OP.mult)
                        nc.vector.tensor_tensor(m2[:], qo, sin_i, OP.mult)
                        nc.vector.tensor_tensor(m3[:], qe, sin_i, OP.mult)
                        nc.vector.tensor_tensor(m4[:], qo, cos_i, OP.mult)
                        nc.vector.tensor_tensor(tgt[:, i, :], m1[:], m2[:], OP.subtract)
                        nc.vector.tensor_tensor(tgt[:, 4 + i, :], m3[:], m4[:], OP.add)

            # ---- stage D: attention in_proj ----
            # dst layout: eo-split halves [128, 4 eo, 1024 t] so score matmuls
            # can take a single [64, 1024] rhs spanning both streams
            qk_sb = {}
            for jj, wd, bb in ((0, wq_d, bq), (1, wk_d, bk)):
                qk_sb[jj] = [big_tile([P, 4, T]) for _ in range(2)]
                dst = qk_sb[jj]
                for qc in range(2):
                    src = qkv[qc][jj]
                    for eo in range(8):
                        wt = w1p.tile([P, 8, P], DTM, tag="w1", name=_nm("wt"))
                        nc.sync.dma_start(out=wt[:], in_=wd[eo])
                        ps = ps_mm()
                        for ko in range(8):
                            nc.tensor.matmul(ps[:], lhsT=wt[:, ko, :], rhs=src[:, ko, :],
                                             start=(ko == 0), stop=(ko == 7))
                        copy_bias(dst[eo // 4][:, eo % 4, qc * S:(qc + 1) * S], ps[:], bb[:, eo:eo + 1])
            q_sb, k_sb = qk_sb[0], qk_sb[1]

            # v' in natural [t, e'] layout, packed per head with ones column
            for ec in range(2):
                for tog in range(2):
                    pss = [ps_mm(), ps_mm(),
                           psum.tile([P, 512], F32, tag="sc", bufs=2, name=_nm("psg")),
                           psum.tile([P, 512], F32, tag="pv", bufs=2, name=_nm("psg"))]
                    for vo in range(8):
                        wt2 = w2p.tile([P, 512], DTM, tag="w2", name=_nm("wt2"))
                        nc.sync.dma_start(out=wt2[:], in_=wvT_d[vo * P:(vo + 1) * P, ec * 512:(ec + 1) * 512])
                        for tl in range(4):
                            tg = tog * 4 + tl
                            s2, ttt = divmod(tg, 4)
                            nc.tensor.matmul(pss[tl][:], lhsT=qkv[s2][2][:, vo, ttt * P:(ttt + 1) * P],
                                             rhs=wt2[:], start=(vo == 0), stop=False)
                    for tl in range(4):
                        nc.tensor.matmul(pss[tl][:], lhsT=onesr[:], rhs=bvrow[:, ec * 512:(ec + 1) * 512],
                                         start=False, stop=True)
                    for tl in range(4):
                        kt = tog * 4 + tl
                        nc.vector.tensor_copy(vsb[ec][:, kt, :, 0:64],
                                              pss[tl][:].rearrange("p (h c) -> p h c", h=8))

            # ---- stage E: attention core (transposed scores, N=1024) ----
            # Tails (1/denom + broadcast + normalize) are emitted one head
            # late so the in-order ACT engine never stalls between exp
            # batches waiting for the current head's pv to finish.
            oT = [big_tile([P, 8, S]) for _ in range(2)]
            pending = []

            def emit_tail(ent):
                qc_, fo_, poff_, op_ = ent
                nc.vector.tensor_tensor(oT[qc_][poff_:poff_ + 64, fo_, :], op_[0:64, :],
                                        onesb[0:64, :], OP.mult)
                lnt = rbp.tile([65, 512], F32, tag="lnt", name=_nm("lnt"), bufs=3)
                nc.scalar.activation(lnt[64:65, :], op_[64:65, :], AF.Ln)
                rct = rbp.tile([65, 512], DTM, tag="rct", name=_nm("rct"), bufs=3)
                with nc.allow_low_precision(reason="softmax 1/denom via exp(-ln d)"):
                    nc.scalar.activation(rct[64:65, :], lnt[64:65, :], AF.Exp, scale=-1.0)
                bp = psum.tile([P, 512], F32, tag="sc", bufs=2, name=_nm("psbc"))
                nc.tensor.matmul(bp[0:64, :], lhsT=onesb[64:65, 0:64], rhs=rct[64:65, :],
                                 start=True, stop=True)
                nc.vector.tensor_tensor(oT[qc_][poff_:poff_ + 64, fo_, :],
                                        oT[qc_][poff_:poff_ + 64, fo_, :], bp[0:64, :], OP.mult)

            for h in range(H):
                vt = vsb[h // 8]
                hh = h % 8
                fo = h // 2
                poff = 64 * (h % 2)
                half, fi = fo // 4, fo % 4
                pT = [big_tile([P, 4, T]) for _ in range(2)]  # kc 0-3, kc 4-7
                for kc in range(8):
                    ps = psum.tile([P, T], F32, tag="sc", bufs=2, name=_nm("pssc"))
                    for qc in range(2):
                        nc.tensor.matmul(
                            ps[:, qc * S:(qc + 1) * S],
                            lhsT=k_sb[half][poff:poff + 64, fi, kc * P:(kc + 1) * P],
                            rhs=q_sb[half][poff:poff + 64, fi, qc * S:(qc + 1) * S],
                            start=True, stop=True)
                    nc.scalar.activation(pT[kc // 4][:, kc % 4, :], ps[:], AF.Exp)
                    # dummy weight loads keep the PE HAM activity monitor warm
                    # through the exp-paced attention window (no PSUM needed)
                    nc.tensor.ldweights(onesb[:, 0:P])
                    nc.tensor.ldweights(onesb[:, 0:P])
                for qc in range(2):
                    hq = h * 2 + qc
                    op = psum.tile([P, 512], F32, tag=("pv" if qc else "mm"),
                                   bufs=2, name=_nm("pspv"))
                    for kc in range(8):
                        nc.tensor.matmul(op[0:65, :], lhsT=vt[:, kc, hh, :],
                                         rhs=pT[kc // 4][:, kc % 4, qc * S:(qc + 1) * S],
                                         start=(kc == 0), stop=(kc == 7))
                    pending.append((qc, fo, poff, op))
                if h > 0:
                    for ent in pending[:2]:
                        emit_tail(ent)
                    pending = pending[2:]
            for ent in pending:
                emit_tail(ent)

            # ---- stage F: out_proj ----
            yT = [big_tile([P, 8, S]) for _ in range(2)]
            for qc in range(2):
                for eo in range(8):
                    wt = w1p.tile([P, 8, P], DTM, tag="w1", name=_nm("wt"))
                    nc.sync.dma_start(out=wt[:], in_=wo_d[eo])
                    ps = ps_mm()
                    for fo in range(8):
                        nc.tensor.matmul(ps[:], lhsT=wt[:, fo, :], rhs=oT[qc][:, fo, :],
                                         start=(fo == 0), stop=(fo == 7))
                    copy_bias(yT[qc][:, eo, :], ps[:], bo[:, eo:eo + 1])

            # ---- MLPs + residual ----
            for s in range(2):
                hts = [big_tile([P, 8, S]) for _ in range(4)]
                for fo in range(32):
                    wt = w1p.tile([P, 8, P], DTM, tag="w1", name=_nm("wt"))
                    nc.sync.dma_start(out=wt[:], in_=w1_d[s][fo])
                    ps = ps_mm()
                    for ko in range(8):
                        nc.tensor.matmul(ps[:], lhsT=wt[:, ko, :], rhs=yT[s][:, ko, :],
                                         start=(ko == 0), stop=(ko == 7))
                    nc.scalar.activation(hts[fo // 8][:, fo % 8, :], ps[:], AF.Gelu_apprx_tanh,
                                         bias=b1[s][:, fo:fo + 1])
                zn2 = znp.tile([P, 4, D], F32, tag="zn", name=_nm("zn2"))
                nc.sync.dma_start(out=zn2[:], in_=z_src[s].rearrange("(to p) d -> p to d", p=P))
                for ec in range(2):
                    pss = [ps_mm(), ps_mm(),
                           psum.tile([P, 512], F32, tag="sc", bufs=2, name=_nm("psg")),
                           psum.tile([P, 512], F32, tag="pv", bufs=2, name=_nm("psg"))]
                    for fo in range(32):
                        wt2 = w2p.tile([P, 512], DTM, tag="w2", name=_nm("wt2"))
                        nc.sync.dma_start(out=wt2[:], in_=w2T_d[s][fo * P:(fo + 1) * P, ec * 512:(ec + 1) * 512])
                        for tl in range(4):
                            nc.tensor.matmul(pss[tl][:], lhsT=hts[fo // 8][:, fo % 8, tl * P:(tl + 1) * P],
                                             rhs=wt2[:], start=(fo == 0), stop=False)
                    for tl in range(4):
                        nc.tensor.matmul(pss[tl][:], lhsT=onesr[:], rhs=b2row[s][:, ec * 512:(ec + 1) * 512],
                                         start=False, stop=True)
                    for tl in range(4):
                        t1 = rs.tile([P, 512], F32, tag="rs", name=_nm("ost"))
                        nc.vector.tensor_tensor(t1[:], pss[tl][:], resb[:, ec * 512:(ec + 1) * 512], OP.mult)
                        t2 = rs.tile([P, 512], F32, tag="rs", name=_nm("ost"))
                        nc.vector.tensor_tensor(t2[:], t1[:], zn2[:, tl, ec * 512:(ec + 1) * 512], OP.add)
                        nc.sync.dma_start(out=out_d[s][tl * P:(tl + 1) * P, ec * 512:(ec + 1) * 512], in_=t2[:])

    nc.finalize()
    return nc


def _to4(WT):
    """WT [Din, Eout] -> [Eout/128, 128p, Din/128, 128e] tiles for lhsT DMA."""
    din, eout = WT.shape
    a = WT.reshape(din // P, P, eout // P, P)       # [ko, p, eo, e]
    return np.ascontiguousarray(a.transpose(2, 1, 0, 3).astype(NPM))


def _bias_part(b, n_tiles):
    return np.ascontiguousarray(b.reshape(n_tiles, P).T)


def _prep_shared(inputs):
    f32 = lambda x: np.ascontiguousarray(np.asarray(x, dtype=np.float32))
    perm = np.concatenate([np.arange(0, D, 2), np.arange(1, D, 2)])

    shared = {}
    for s, (wn, bn) in enumerate((("qkv_state_w", "qkv_state_b"), ("qkv_action_w", "qkv_action_b"))):
        w = f32(inputs[wn])
        b = f32(inputs[bn])
        wp = np.concatenate([w[0:D][perm], w[D:2 * D][perm], w[2 * D:3 * D]], axis=0)
        bp = np.concatenate([b[0:D][perm], b[D:2 * D][perm], b[2 * D:3 * D]])
        shared[f"wqkv{s}"] = _to4(wp.T)
        shared[f"bqkv{s}"] = _bias_part(bp, 24)

    in_w = f32(inputs["attn_in_w"])
    in_b = f32(inputs["attn_in_b"])
    wq, wk, wv = in_w[0:D], in_w[D:2 * D], in_w[2 * D:3 * D]
    bq_, bk_, bv_ = in_b[0:D], in_b[D:2 * D], in_b[2 * D:3 * D]
    scale = 1.0 / math.sqrt(HD)
    shared["wq"] = _to4((wq[:, perm] * scale).T)
    shared["bq"] = _bias_part(bq_ * scale, 8)
    shared["wk"] = _to4(wk[:, perm].T)
    shared["bk"] = _bias_part(bk_, 8)
    shared["wvT"] = np.ascontiguousarray(wv.T.astype(NPM))
    shared["bvrow"] = np.ascontiguousarray(bv_[None, :].astype(NPM))
    shared["wo"] = _to4(f32(inputs["attn_out_w"]).T)
    shared["bo"] = _bias_part(f32(inputs["attn_out_b"]), 8)
    for s, pre in enumerate(("mlp_state", "mlp_action")):
        shared[f"w1{s}"] = _to4(f32(inputs[f"{pre}_w1"]).T)
        shared[f"b1{s}"] = _bias_part(f32(inputs[f"{pre}_b1"]), 32)
        shared[f"w2T{s}"] = np.ascontiguousarray(f32(inputs[f"{pre}_w2"]).T.astype(NPM))
        shared[f"b2row{s}"] = np.ascontiguousarray(f32(inputs[f"{pre}_b2"])[None, :].astype(NPM))

    inv = np.exp(-math.log(MAX_LEN) * np.arange(0, D, 2, dtype=np.float64) / D)
    theta = inv[:, None] * np.arange(S, dtype=np.float64)[None, :]   # [i, t]
    cosT = np.cos(theta).astype(np.float32)
    sinT = np.sin(theta).astype(np.float32)
    shared["cosT"] = np.ascontiguousarray(cosT.reshape(4, P, S).transpose(1, 0, 2).reshape(P, 2048).astype(NPM))
    shared["sinT"] = np.ascontiguousarray(sinT.reshape(4, P, S).transpose(1, 0, 2).reshape(P, 2048).astype(NPM))
    shared["ident"] = np.eye(P, dtype=np.float32)
    shared["onesr"] = np.ones((1, P), NPM)
    shared["onesc"] = np.ones((P, 16, 1), NPM)
    shared["onesb"] = np.ones((P, 512), NPM)
    return shared


def _prep_in_maps(inputs):
    f32 = lambda x: np.ascontiguousarray(np.asarray(x, dtype=np.float32))
    shared = _prep_shared(inputs)
    state_z = f32(inputs["state_z"])
    action_z = f32(inputs["action_z"])
    e = f32(inputs["e"])
    in_maps = []
    for b in range(B):
        shift = e[b, 0, 0:D]
        scl = e[b, 0, D:2 * D]
        res = e[b, 0, 2 * D:3 * D]
        m = dict(shared)
        m["sz"] = state_z[b]
        m["az"] = action_z[b]
        m["scale1p"] = _bias_part(1.0 + scl, 8)
        m["shift"] = _bias_part(shift, 8)
        m["resb"] = np.ascontiguousarray(np.broadcast_to(res[None, :], (P, D)))
        in_maps.append(m)
    return in_maps


def _run(inputs, trace=False, trace_kwargs=None):
    key = "nc"
    if key not in _BUILD_CACHE:
        _BUILD_CACHE[key] = _build_nc()
    nc = _BUILD_CACHE[key]
    in_maps = _prep_in_maps(inputs)
    kw = {}
    if trace:
        kw = dict(trace=True, trace_kwargs=trace_kwargs or {})
    return run_bass_kernel_spmd(nc, in_maps, list(range(N_CORES)), **kw)


def kernel(**inputs):
    res = _run(inputs)
    state = np.stack([res.results[b]["state_out"] for b in range(B)])
    action = np.stack([res.results[b]["action_out"] for b in range(B)])
    return (state, action)


def kernel_timed(**inputs):
    """Returns ((state, action), exec_time_ns) using the NTFF profile path."""
    res = _run(inputs, trace=True)
    state = np.stack([res.results[b]["state_out"] for b in range(B)])
    action = np.stack([res.results[b]["action_out"] for b in range(B)])
    return (state, action), res.exec_time_ns
